# revision 12
# baseline (speedup 1.0000x reference)
"""Trainium2 Bass kernel for HAINT attention (nn_HAINT_Attention_77094662963332).

Reference computation (B=256, T=512, H=512):
    q   = concat(ht, ct)                       # [B, 2H]
    a_s = q @ W_as                             # [B, H]
    ah  = hi @ W_ah                            # [B, T, H]
    etk = tanh(a_s[:,None,:] + ah + ba)        # [B, T, H]
    etk = etk @ W_a                            # [B, T, H]
    atk = softmax(etk, axis=1)                 # softmax over T
    e   = sum(atk * hi, axis=1)                # [B, H]

Strategy: data-parallel over B across 8 cores (32 batches/core). Per batch,
work in a transposed layout ([h or k] on partitions, t on the free dim):
    Xb^T = hi[b]^T (bf16, via hardware DMA-transpose of the natural-layout load)
    etk^T[k,t]  = tanh(sum_h W_ah[h,k] * X^T[h,t] + biasT[k,b])   (PE + ACT)
    etk2^T[k,t] = sum_h W_a[h,k] * etk^T[h,t]                     (PE)
    p = exp(etk2^T)          with accum_out -> den[k,b]           (ACT)
    num[k,b] = sum_t p*X^T   via fused tensor_tensor_reduce       (DVE)
    e^T[k,b] = num/den                                            (DVE)
Softmax max-subtraction is skipped: logits are bounded (|etk|<=1, W_a ~ 0.05
normal), so exp stays comfortably within fp32 range and matches the
reference to fp rounding.

Output is produced transposed ([H, B_loc] per core) and transposed back on
host during the gather.
"""

import os
import sys

import numpy as np

for _p in ("/opt/trn_rl_repo",):
    if _p not in sys.path and os.path.isdir(_p):
        sys.path.insert(0, _p)

B, T, H = 256, 512, 512
N_CORES = 8
B_LOC = B // N_CORES  # 32
PB = 128  # partition block
TB = T // PB  # 4 t-blocks
HB = H // PB  # 4 h-blocks
KB = H // PB  # 4 k-blocks
QB = 2 * H // PB  # 8 q-blocks


def build_bass():
    import concourse.bass as bass  # noqa: F401
    import concourse.mybir as mybir
    import concourse.tile as tile
    from concourse import bacc
    from concourse.masks import make_identity

    f32 = mybir.dt.float32
    bf16 = mybir.dt.bfloat16
    AF = mybir.ActivationFunctionType
    ALU = mybir.AluOpType

    nc = bacc.Bacc(None, target_bir_lowering=False)

    ht = nc.declare_dram_parameter("ht", [B_LOC, H], f32, isOutput=False)
    ct = nc.declare_dram_parameter("ct", [B_LOC, H], f32, isOutput=False)
    hi = nc.declare_dram_parameter("hi", [B_LOC, T, H], f32, isOutput=False)
    W_as = nc.declare_dram_parameter("W_as", [2 * H, H], f32, isOutput=False)
    W_ah = nc.declare_dram_parameter("W_ah", [H, H], f32, isOutput=False)
    ba = nc.declare_dram_parameter("ba", [1, H], f32, isOutput=False)
    W_a = nc.declare_dram_parameter("W_a", [H, H], f32, isOutput=False)
    eT = nc.declare_dram_parameter("eT", [H, B_LOC], f32, isOutput=True)

    with tile.TileContext(nc) as tc:
        with (
            tc.tile_pool(name="consts", bufs=1) as consts,
            tc.tile_pool(name="setup_psum", bufs=1, space="PSUM") as setup_psum,
            tc.tile_pool(name="hib_pool", bufs=6, space="DRAM") as hib_pool,
            tc.tile_pool(name="xt_pool", bufs=6) as xt_pool,
            tc.tile_pool(name="etk_pool", bufs=3) as etk_pool,
            tc.tile_pool(name="p_pool", bufs=4) as p_pool,
            tc.tile_pool(name="prod_pool", bufs=4) as prod_pool,
            tc.tile_pool(name="fin_pool", bufs=2) as fin_pool,
            tc.tile_pool(name="ps1_pool", bufs=3, space="PSUM") as ps1_pool,
            tc.tile_pool(name="ps2_pool", bufs=3, space="PSUM") as ps2_pool,
        ):
            # ---------------- setup: weights (cast to bf16 on the fly) -------
            # W_ah[h,k] tiles: partition = h within block, free = (hb, k).
            wah_sb = consts.tile([PB, HB, H], bf16)
            nc.gpsimd.dma_start(
                out=wah_sb, in_=W_ah[:, :].rearrange("(hb p) k -> p hb k", p=PB)
            )
            wa_sb = consts.tile([PB, HB, H], bf16)
            nc.gpsimd.dma_start(
                out=wa_sb, in_=W_a[:, :].rearrange("(hb p) k -> p hb k", p=PB)
            )
            was_sb = consts.tile([PB, QB, H], bf16)
            nc.gpsimd.dma_start(
                out=was_sb, in_=W_as[:, :].rearrange("(qb p) k -> p qb k", p=PB)
            )
            ba_sb = consts.tile([1, H], bf16)
            nc.gpsimd.dma_start(out=ba_sb, in_=ba[:, :])
            ones_sb = consts.tile([1, B_LOC], bf16)
            nc.vector.memset(ones_sb, 1.0)

            ht_sb = consts.tile([B_LOC, H], bf16)
            nc.gpsimd.dma_start(out=ht_sb, in_=ht[:, :])
            ct_sb = consts.tile([B_LOC, H], bf16)
            nc.gpsimd.dma_start(out=ct_sb, in_=ct[:, :])

            ident = consts.tile([B_LOC, B_LOC], bf16)
            make_identity(nc, ident)

            # qT[q, b] (q = concat feature dim, 8 blocks of 128) via PE transpose.
            qT_sb = consts.tile([PB, QB, B_LOC], bf16)
            for i, src in enumerate((ht_sb, ct_sb)):
                for j in range(HB):
                    ps_t = setup_psum.tile([PB, B_LOC], bf16, tag="ps_t")
                    nc.tensor.transpose(
                        ps_t, src[:, j * PB : (j + 1) * PB], ident
                    )
                    nc.vector.tensor_copy(out=qT_sb[:, i * HB + j, :], in_=ps_t)

            # biasT[k, b] = (q @ W_as)^T + ba^T, computed as
            #   sum_qb W_as_block^T @ qT_block  +  ba_block^T @ ones
            biasT = consts.tile([PB, KB, B_LOC], f32)
            for kb in range(KB):
                ps_as = setup_psum.tile([PB, B_LOC], f32, tag="ps_as")
                for qb in range(QB):
                    nc.tensor.matmul(
                        ps_as,
                        lhsT=was_sb[:, qb, kb * PB : (kb + 1) * PB],
                        rhs=qT_sb[:, qb, :],
                        start=(qb == 0),
                        stop=False,
                    )
                nc.tensor.matmul(
                    ps_as,
                    lhsT=ba_sb[:, kb * PB : (kb + 1) * PB],
                    rhs=ones_sb,
                    start=False,
                    stop=True,
                )
                nc.vector.tensor_copy(out=biasT[:, kb, :], in_=ps_as)

            den_st = consts.tile([PB, KB, B_LOC], f32)
            num_st = consts.tile([PB, KB, B_LOC], f32)

            # ---------------- main loop over local batches -------------------
            for b in range(B_LOC):
                # Stage hi[b] as bf16 in DRAM (SWDGE cast copy, DRAM->DRAM),
                # then transpose straight from DRAM with 4 big xbar-transpose
                # DMAs ([512,128] -> [128,512]) — far fewer, larger transpose
                # ops than SBUF->SBUF 128x128 tiling (which measured ~1.25us
                # per op, serialized, and starved the PE cold).
                hib = hib_pool.tile([T, H], bf16, tag="hib", space="DRAM")
                nc.gpsimd.dma_start(out=hib, in_=hi[b, :, :])

                xt = xt_pool.tile([PB, HB, T], bf16, tag="xt")
                for hb in range(HB):
                    nc.sync.dma_start(
                        out=xt[:, hb, :],
                        in_=hib[:, hb * PB : (hb + 1) * PB],
                        transpose=True,
                    )

                # mm1 + tanh -> etk^T (bf16)
                etk = etk_pool.tile([PB, KB, T], bf16, tag="etk")
                for kb in range(KB):
                    ps1 = ps1_pool.tile([PB, T], f32, tag="ps1")
                    for hb in range(HB):
                        nc.tensor.matmul(
                            ps1,
                            lhsT=wah_sb[:, hb, kb * PB : (kb + 1) * PB],
                            rhs=xt[:, hb, :],
                            start=(hb == 0),
                            stop=(hb == HB - 1),
                        )
                    nc.scalar.activation(
                        out=etk[:, kb, :],
                        in_=ps1,
                        func=AF.Tanh,
                        bias=biasT[:, kb, b : b + 1],
                        scale=1.0,
                    )

                # mm2 + exp (accumulate denominator) + fused mul-reduce numerator
                for kb in range(KB):
                    ps2 = ps2_pool.tile([PB, T], f32, tag="ps2")
                    for hb in range(HB):
                        nc.tensor.matmul(
                            ps2,
                            lhsT=wa_sb[:, hb, kb * PB : (kb + 1) * PB],
                            rhs=etk[:, hb, :],
                            start=(hb == 0),
                            stop=(hb == HB - 1),
                        )
                    p = p_pool.tile([PB, T], bf16, tag="p")
                    nc.scalar.activation(
                        out=p,
                        in_=ps2,
                        func=AF.Exp,
                        accum_out=den_st[:, kb, b : b + 1],
                    )
                    # (tensor_tensor_reduce would fuse these, but it crashes
                    # this runtime's DVE — NRT_EXEC_UNIT_UNRECOVERABLE)
                    prod = prod_pool.tile([PB, T], bf16, tag="prod")
                    nc.vector.tensor_mul(prod, p, xt[:, kb, :])
                    nc.vector.tensor_reduce(
                        out=num_st[:, kb, b : b + 1],
                        in_=prod,
                        axis=mybir.AxisListType.X,
                        op=ALU.add,
                    )

            # ---------------- finalize: e^T = num / den ----------------------
            for kb in range(KB):
                rden = fin_pool.tile([PB, B_LOC], f32, tag="rden")
                nc.vector.reciprocal(rden, den_st[:, kb, :])
                eT_sb = fin_pool.tile([PB, B_LOC], f32, tag="eT_sb")
                nc.vector.tensor_mul(eT_sb, num_st[:, kb, :], rden)
                nc.sync.dma_start(out=eT[kb * PB : (kb + 1) * PB, :], in_=eT_sb)

    nc.compile()
    return nc


def run(inputs, trace=False):
    """Run on 8 cores. inputs: dict of full-size numpy arrays. Returns
    (full_output [B,H] f32, BassKernelResults)."""
    from concourse.bass_utils import run_bass_kernel_spmd

    nc = build_bass()

    ht = np.ascontiguousarray(np.asarray(inputs["ht"], dtype=np.float32))
    ct = np.ascontiguousarray(np.asarray(inputs["ct"], dtype=np.float32))
    hi = np.ascontiguousarray(np.asarray(inputs["hi"], dtype=np.float32))
    W_as = np.ascontiguousarray(np.asarray(inputs["W_as"], dtype=np.float32))
    W_ah = np.ascontiguousarray(np.asarray(inputs["W_ah"], dtype=np.float32))
    ba = np.ascontiguousarray(np.asarray(inputs["ba"], dtype=np.float32))
    W_a = np.ascontiguousarray(np.asarray(inputs["W_a"], dtype=np.float32))

    in_maps = []
    for c in range(N_CORES):
        sl = slice(c * B_LOC, (c + 1) * B_LOC)
        in_maps.append(
            {
                "ht": np.ascontiguousarray(ht[sl]),
                "ct": np.ascontiguousarray(ct[sl]),
                "hi": np.ascontiguousarray(hi[sl]),
                "W_as": W_as,
                "W_ah": W_ah,
                "ba": ba,
                "W_a": W_a,
            }
        )

    res = run_bass_kernel_spmd(nc, in_maps, core_ids=list(range(N_CORES)), trace=trace)
    out = np.concatenate([r["eT"].T for r in res.results], axis=0)
    return np.ascontiguousarray(out.astype(np.float32)), res


def kernel(**inputs) -> np.ndarray:
    out, _ = run(inputs, trace=False)
    return out


# revision 14
# speedup vs baseline: 1.2038x; 1.2038x over previous
"""Trainium2 Bass kernel for HAINT attention (nn_HAINT_Attention_77094662963332).

Reference computation (B=256, T=512, H=512):
    q   = concat(ht, ct)                       # [B, 2H]
    a_s = q @ W_as                             # [B, H]
    ah  = hi @ W_ah                            # [B, T, H]
    etk = tanh(a_s[:,None,:] + ah + ba)        # [B, T, H]
    etk = etk @ W_a                            # [B, T, H]
    atk = softmax(etk, axis=1)                 # softmax over T
    e   = sum(atk * hi, axis=1)                # [B, H]

Strategy: data-parallel over B across 8 cores (32 batches/core). Per batch,
work in a transposed layout ([h or k] on partitions, t on the free dim):
    Xb^T = hi[b]^T (bf16, via hardware DMA-transpose of the natural-layout load)
    etk^T[k,t]  = tanh(sum_h W_ah[h,k] * X^T[h,t] + biasT[k,b])   (PE + ACT)
    etk2^T[k,t] = sum_h W_a[h,k] * etk^T[h,t]                     (PE)
    p = exp(etk2^T)          with accum_out -> den[k,b]           (ACT)
    num[k,b] = sum_t p*X^T   via fused tensor_tensor_reduce       (DVE)
    e^T[k,b] = num/den                                            (DVE)
Softmax max-subtraction is skipped: logits are bounded (|etk|<=1, W_a ~ 0.05
normal), so exp stays comfortably within fp32 range and matches the
reference to fp rounding.

Output is produced transposed ([H, B_loc] per core) and transposed back on
host during the gather.
"""

import os
import sys

import numpy as np

for _p in ("/opt/trn_rl_repo",):
    if _p not in sys.path and os.path.isdir(_p):
        sys.path.insert(0, _p)

B, T, H = 256, 512, 512
N_CORES = 8
B_LOC = B // N_CORES  # 32
PB = 128  # partition block
TB = T // PB  # 4 t-blocks
HB = H // PB  # 4 h-blocks
KB = H // PB  # 4 k-blocks
QB = 2 * H // PB  # 8 q-blocks


def build_bass():
    import concourse.bass as bass  # noqa: F401
    import concourse.mybir as mybir
    import concourse.tile as tile
    from concourse import bacc
    from concourse.masks import make_identity

    f32 = mybir.dt.float32
    bf16 = mybir.dt.bfloat16
    AF = mybir.ActivationFunctionType
    ALU = mybir.AluOpType

    nc = bacc.Bacc(None, target_bir_lowering=False)

    ht = nc.declare_dram_parameter("ht", [B_LOC, H], f32, isOutput=False)
    ct = nc.declare_dram_parameter("ct", [B_LOC, H], f32, isOutput=False)
    hi = nc.declare_dram_parameter("hi", [B_LOC, T, H], f32, isOutput=False)
    W_as = nc.declare_dram_parameter("W_as", [2 * H, H], f32, isOutput=False)
    W_ah = nc.declare_dram_parameter("W_ah", [H, H], f32, isOutput=False)
    ba = nc.declare_dram_parameter("ba", [1, H], f32, isOutput=False)
    W_a = nc.declare_dram_parameter("W_a", [H, H], f32, isOutput=False)
    eT = nc.declare_dram_parameter("eT", [H, B_LOC], f32, isOutput=True)

    with tile.TileContext(nc) as tc:
        with (
            tc.tile_pool(name="consts", bufs=1) as consts,
            tc.tile_pool(name="setup_psum", bufs=1, space="PSUM") as setup_psum,
            tc.tile_pool(name="hib_pool", bufs=2, space="DRAM") as hib_pool,
            tc.tile_pool(name="xt_pool", bufs=3) as xt_pool,
            tc.tile_pool(name="etk_pool", bufs=3) as etk_pool,
            tc.tile_pool(name="p_pool", bufs=4) as p_pool,
            tc.tile_pool(name="prod_pool", bufs=4) as prod_pool,
            tc.tile_pool(name="fin_pool", bufs=2) as fin_pool,
            tc.tile_pool(name="ps1_pool", bufs=3, space="PSUM") as ps1_pool,
            tc.tile_pool(name="ps2_pool", bufs=3, space="PSUM") as ps2_pool,
        ):
            # ---------------- setup: weights (cast to bf16 on the fly) -------
            # W_ah[h,k] tiles: partition = h within block, free = (hb, k).
            wah_sb = consts.tile([PB, HB, H], bf16)
            nc.gpsimd.dma_start(
                out=wah_sb, in_=W_ah[:, :].rearrange("(hb p) k -> p hb k", p=PB)
            )
            wa_sb = consts.tile([PB, HB, H], bf16)
            nc.gpsimd.dma_start(
                out=wa_sb, in_=W_a[:, :].rearrange("(hb p) k -> p hb k", p=PB)
            )
            was_sb = consts.tile([PB, QB, H], bf16)
            nc.gpsimd.dma_start(
                out=was_sb, in_=W_as[:, :].rearrange("(qb p) k -> p qb k", p=PB)
            )
            ba_sb = consts.tile([1, H], bf16)
            nc.gpsimd.dma_start(out=ba_sb, in_=ba[:, :])
            ones_sb = consts.tile([1, B_LOC], bf16)
            nc.vector.memset(ones_sb, 1.0)

            ht_sb = consts.tile([B_LOC, H], bf16)
            nc.gpsimd.dma_start(out=ht_sb, in_=ht[:, :])
            ct_sb = consts.tile([B_LOC, H], bf16)
            nc.gpsimd.dma_start(out=ct_sb, in_=ct[:, :])

            ident = consts.tile([B_LOC, B_LOC], bf16)
            make_identity(nc, ident)

            # qT[q, b] (q = concat feature dim, 8 blocks of 128) via PE transpose.
            qT_sb = consts.tile([PB, QB, B_LOC], bf16)
            for i, src in enumerate((ht_sb, ct_sb)):
                for j in range(HB):
                    ps_t = setup_psum.tile([PB, B_LOC], bf16, tag="ps_t")
                    nc.tensor.transpose(
                        ps_t, src[:, j * PB : (j + 1) * PB], ident
                    )
                    nc.vector.tensor_copy(out=qT_sb[:, i * HB + j, :], in_=ps_t)

            # biasT[k, b] = (q @ W_as)^T + ba^T, computed as
            #   sum_qb W_as_block^T @ qT_block  +  ba_block^T @ ones
            biasT = consts.tile([PB, KB, B_LOC], f32)
            for kb in range(KB):
                ps_as = setup_psum.tile([PB, B_LOC], f32, tag="ps_as")
                for qb in range(QB):
                    nc.tensor.matmul(
                        ps_as,
                        lhsT=was_sb[:, qb, kb * PB : (kb + 1) * PB],
                        rhs=qT_sb[:, qb, :],
                        start=(qb == 0),
                        stop=False,
                    )
                nc.tensor.matmul(
                    ps_as,
                    lhsT=ba_sb[:, kb * PB : (kb + 1) * PB],
                    rhs=ones_sb,
                    start=False,
                    stop=True,
                )
                nc.vector.tensor_copy(out=biasT[:, kb, :], in_=ps_as)

            den_st = consts.tile([PB, KB, B_LOC], f32)
            num_st = consts.tile([PB, KB, B_LOC], f32)

            # ---------------- main loop over local batches -------------------
            for b in range(B_LOC):
                # Stage hi[b] as bf16 in DRAM (SWDGE cast copy, DRAM->DRAM),
                # then transpose straight from DRAM with 4 big xbar-transpose
                # DMAs ([512,128] -> [128,512]) — far fewer, larger transpose
                # ops than SBUF->SBUF 128x128 tiling (which measured ~1.25us
                # per op, serialized, and starved the PE cold).
                hib = hib_pool.tile([T, H], bf16, tag="hib", space="DRAM")
                nc.gpsimd.dma_start(out=hib, in_=hi[b, :, :])

                xt = xt_pool.tile([PB, HB, T], bf16, tag="xt")
                for hb in range(HB):
                    nc.sync.dma_start(
                        out=xt[:, hb, :],
                        in_=hib[:, hb * PB : (hb + 1) * PB],
                        transpose=True,
                    )

                # mm1 + tanh -> etk^T (bf16)
                etk = etk_pool.tile([PB, KB, T], bf16, tag="etk")
                for kb in range(KB):
                    ps1 = ps1_pool.tile([PB, T], f32, tag="ps1")
                    for hb in range(HB):
                        nc.tensor.matmul(
                            ps1,
                            lhsT=wah_sb[:, hb, kb * PB : (kb + 1) * PB],
                            rhs=xt[:, hb, :],
                            start=(hb == 0),
                            stop=(hb == HB - 1),
                        )
                    nc.scalar.activation(
                        out=etk[:, kb, :],
                        in_=ps1,
                        func=AF.Tanh,
                        bias=biasT[:, kb, b : b + 1],
                        scale=1.0,
                    )

                # mm2 + exp (accumulate denominator) + fused mul-reduce numerator
                for kb in range(KB):
                    ps2 = ps2_pool.tile([PB, T], f32, tag="ps2")
                    for hb in range(HB):
                        nc.tensor.matmul(
                            ps2,
                            lhsT=wa_sb[:, hb, kb * PB : (kb + 1) * PB],
                            rhs=etk[:, hb, :],
                            start=(hb == 0),
                            stop=(hb == HB - 1),
                        )
                    p = p_pool.tile([PB, T], bf16, tag="p")
                    nc.scalar.activation(
                        out=p,
                        in_=ps2,
                        func=AF.Exp,
                        accum_out=den_st[:, kb, b : b + 1],
                    )
                    # (tensor_tensor_reduce would fuse these, but it crashes
                    # this runtime's DVE — NRT_EXEC_UNIT_UNRECOVERABLE)
                    prod = prod_pool.tile([PB, T], bf16, tag="prod")
                    nc.vector.tensor_mul(prod, p, xt[:, kb, :])
                    nc.vector.tensor_reduce(
                        out=num_st[:, kb, b : b + 1],
                        in_=prod,
                        axis=mybir.AxisListType.X,
                        op=ALU.add,
                    )

            # ---------------- finalize: e^T = num / den ----------------------
            for kb in range(KB):
                rden = fin_pool.tile([PB, B_LOC], f32, tag="rden")
                nc.vector.reciprocal(rden, den_st[:, kb, :])
                eT_sb = fin_pool.tile([PB, B_LOC], f32, tag="eT_sb")
                nc.vector.tensor_mul(eT_sb, num_st[:, kb, :], rden)
                nc.sync.dma_start(out=eT[kb * PB : (kb + 1) * PB, :], in_=eT_sb)

    nc.compile()
    return nc


def run(inputs, trace=False):
    """Run on 8 cores. inputs: dict of full-size numpy arrays. Returns
    (full_output [B,H] f32, BassKernelResults)."""
    from concourse.bass_utils import run_bass_kernel_spmd

    nc = build_bass()

    ht = np.ascontiguousarray(np.asarray(inputs["ht"], dtype=np.float32))
    ct = np.ascontiguousarray(np.asarray(inputs["ct"], dtype=np.float32))
    hi = np.ascontiguousarray(np.asarray(inputs["hi"], dtype=np.float32))
    W_as = np.ascontiguousarray(np.asarray(inputs["W_as"], dtype=np.float32))
    W_ah = np.ascontiguousarray(np.asarray(inputs["W_ah"], dtype=np.float32))
    ba = np.ascontiguousarray(np.asarray(inputs["ba"], dtype=np.float32))
    W_a = np.ascontiguousarray(np.asarray(inputs["W_a"], dtype=np.float32))

    in_maps = []
    for c in range(N_CORES):
        sl = slice(c * B_LOC, (c + 1) * B_LOC)
        in_maps.append(
            {
                "ht": np.ascontiguousarray(ht[sl]),
                "ct": np.ascontiguousarray(ct[sl]),
                "hi": np.ascontiguousarray(hi[sl]),
                "W_as": W_as,
                "W_ah": W_ah,
                "ba": ba,
                "W_a": W_a,
            }
        )

    res = run_bass_kernel_spmd(nc, in_maps, core_ids=list(range(N_CORES)), trace=trace)
    out = np.concatenate([r["eT"].T for r in res.results], axis=0)
    return np.ascontiguousarray(out.astype(np.float32)), res


def kernel(**inputs) -> np.ndarray:
    out, _ = run(inputs, trace=False)
    return out
